# revision 19
# baseline (speedup 1.0000x reference)
"""Trainium2 Bass kernel for: out = 0.5 * sum_g maxpool4(x @ W.T + b).

Shapes: x [4096, 2048] f32, W [4096, 2048] f32, b [4096] f32 -> out [4096] f32.

Sharding over 8 NeuronCores: 2 batch-groups x 4 out-feature-groups.
Core c = (g, j): batch rows g*2048:(g+1)*2048, out features j*1024:(j+1)*1024.
Each core computes partial row-sums of its pooled quarter; host adds the 4
out-feature partials per batch half (pooling groups of 4 are never split
across cores since 1024 % 4 == 0).

Per-core kernel: y tile layout [batch=128 partitions, out_f=512 free].
  lhsT (stationary) = x^T k-slice [128 i, 128 b], rhs (moving) = W^T k-slice
  [128 i, 512 o], accumulating over 16 k-slices into PSUM fp32. The bias add
  rides the VectorE pooling stage (bias+maxpool4 via strided adds + max
  tree), keeping the PE stream pure matmul. The 0.5 output scale is folded
  into W and b on the host (max is monotone under positive scaling). Inputs
  are cast to bf16 on host (PE runs bf16 at 1 cycle/row vs 4 for fp32); PSUM
  accumulation stays fp32 and the bias is added in fp32.

Loop order: k-major over groups of 4 batch-tiles (8 PSUM banks = 4 b x 2 o)
so each W^T k-slice DMA is consumed by 8 back-to-back matmuls and the PE
never waits on HBM after the first slice. Everything (~12.5 MiB/core) stays
resident in SBUF. The first (xt, wt) slices are DMA'd from GpSimd, which
exits the NEFF preamble ~1.3 us before Sync, shaving the first-matmul
latency; Sync streams the remaining W slices in parallel.
"""

import sys

if "/opt/trn_rl_repo" not in sys.path:
    sys.path.insert(0, "/opt/trn_rl_repo")

import numpy as np
import ml_dtypes

# Problem constants (hardcoded per harness contract).
B, I, O = 4096, 2048, 4096
KS = 4  # maxpool kernel size
SCALE = 0.5
NB_G, NO_G = 2, 4  # batch groups x out-feature groups = 8 cores
BC = B // NB_G  # 2048 batch rows per core
OC = O // NO_G  # 1024 out features per core
P = 128
KT = I // P  # 16 contraction slices
BT = BC // P  # 16 batch tiles per core
GRP = 4  # batch tiles per k-major group
NG = BT // GRP  # 4 groups
NFREE = 512  # matmul moving free dim (one PSUM bank fp32)
OT = OC // NFREE  # 2 out-feature tiles of 512 per core

_NC_CACHE = {}


def _dedup_ldweights(nc):
    """Remove redundant standalone Ldweights from the compiled module.

    bacc splits every Matmult into Ldweights + Matmult(ldweights=False) with
    no dedup, costing ~46 ns/matmul of PE queue time. When consecutive PE
    Ldweights load the identical stationary AP, the array already holds the
    weights, so sync-free duplicates can be dropped. Only duplicates with no
    semaphore waits/updates are removed (a wait-carrying Ldweights guards a
    real dependency).
    """
    removed = 0
    for f in nc.m.functions:
        for blk in f.blocks:
            insts = list(blk.instructions)
            keep = []
            blk_removed = 0
            last_key = None
            for ins in insts:
                tname = type(ins).__name__
                if tname == "InstLdweights":
                    ap = ins.ins[0]
                    key = (
                        ap.memref,
                        ap.offset,
                        str(ap.ap),
                        str(ap.dtype),
                        str(ins.tile_position),
                        str(ins.tile_size),
                        str(ins.perf_mode),
                        str(ins.is_transpose),
                    )
                    if (
                        key == last_key
                        and not ins.has_wait()
                        and not ins.has_update()
                    ):
                        blk_removed += 1
                        continue
                    last_key = key
                keep.append(ins)
            if blk_removed:
                blk.instructions[:] = keep
                removed += blk_removed
    return removed


def build_bass():
    """Build the (SPMD, per-core) Bass program."""
    from concourse import bacc, tile
    import concourse.mybir as mybir

    f32 = mybir.dt.float32
    bf16 = mybir.dt.bfloat16

    # Bacc (not plain Bass): its compile() runs the TRN2 legalization passes
    # (move_matmul_waits_to_ldweights, generate_event_semaphores) without
    # which walrus rejects matmuls carrying >1 semaphore wait.
    nc = bacc.Bacc(
        "TRN2", target_bir_lowering=False, debug=False, num_devices=NB_G * NO_G
    )
    xt_d = nc.dram_tensor("xt", [KT, NG, P, GRP * P], bf16, kind="ExternalInput")
    wt_d = nc.dram_tensor("wt", [KT, P, OC], bf16, kind="ExternalInput")
    biasrep_d = nc.dram_tensor("biasrep", [P, OC], f32, kind="ExternalInput")
    out_d = nc.dram_tensor("out", [P, BT], f32, kind="ExternalOutput")

    with tile.TileContext(nc) as tc:
        with (
            tc.tile_pool(name="wt", bufs=KT) as wt_pool,
            tc.tile_pool(name="xt", bufs=KT * NG) as xt_pool,
            tc.tile_pool(name="misc", bufs=1) as misc_pool,
            tc.tile_pool(name="pool4", bufs=8) as pool4_pool,
            tc.tile_pool(name="pooled", bufs=4) as pooled_pool,
            tc.tile_pool(name="psum", bufs=8, space="PSUM") as psum_pool,
        ):
            # wt streams on Sync (wt0 first — first matmul gates on it), xt
            # on GpSimd, so the two engines' descriptor prep and transfer
            # queues run in parallel.
            wt_sb = []
            xt_sb = {}
            # k=0 arrives in fine-grained pieces so the first matmul (which
            # gates the whole PE stream behind the ~7 us NEFF preamble) can
            # start after ~160 KB instead of ~384 KB: first the bb=0 x
            # columns and the o=0 W half, then the rest.
            w = wt_pool.tile([P, OC], bf16, tag="wt", name="w_0")
            xx = xt_pool.tile([P, GRP * P], bf16, tag="xt", name="xx_0_0")
            nc.gpsimd.dma_start(xx[:, 0:P], xt_d[0, 0, :, 0:P])
            nc.sync.dma_start(w[:, 0:NFREE], wt_d[0, :, 0:NFREE])
            nc.gpsimd.dma_start(xx[:, P:], xt_d[0, 0, :, P:])
            nc.sync.dma_start(w[:, NFREE:], wt_d[0, :, NFREE:])
            wt_sb.append(w)
            xt_sb[(0, 0)] = xx

            biasrep = misc_pool.tile([P, OC], f32)
            nc.scalar.dma_start(biasrep[:], biasrep_d[:, :])
            outsb = misc_pool.tile([P, BT], f32)

            for k in range(1, KT):
                w = wt_pool.tile([P, OC], bf16, tag="wt", name=f"w_{k}")
                nc.sync.dma_start(w[:], wt_d[k, :, :])
                wt_sb.append(w)
                xx = xt_pool.tile([P, GRP * P], bf16, tag="xt", name=f"xx_{k}_0")
                nc.gpsimd.dma_start(xx[:], xt_d[k, 0, :, :])
                xt_sb[(k, 0)] = xx
            for g in range(1, NG):
                for k in range(KT):
                    xx = xt_pool.tile([P, GRP * P], bf16, tag="xt", name=f"xx_{k}_{g}")
                    nc.gpsimd.dma_start(xx[:], xt_d[k, g, :, :])
                    xt_sb[(k, g)] = xx

            def emit_pooling(g, bb, psums):
                """bias add + maxpool4 + row-sum for one finished batch tile."""
                pooled = pooled_pool.tile(
                    [P, OT, P], f32, tag="pooled", name=f"pooled_{g}_{bb}"
                )
                for o in range(OT):
                    # y+bias in one full-width add (frees the PSUM bank
                    # early), then maxpool4 via 3D-AP reduce.
                    tsum = pool4_pool.tile(
                        [P, NFREE], f32, tag="tsum", name=f"tsum_{g}_{bb}_{o}"
                    )
                    nc.vector.tensor_add(
                        tsum[:],
                        psums[bb][o][:],
                        biasrep[:, o * NFREE : (o + 1) * NFREE],
                    )
                    nc.vector.reduce_max(
                        pooled[:, o, :],
                        tsum[:].rearrange("p (q f) -> p q f", f=KS),
                        axis=mybir.AxisListType.X,
                    )
                col = g * GRP + bb
                nc.vector.reduce_sum(
                    outsb[:, col : col + 1],
                    pooled[:, :, :],
                    axis=mybir.AxisListType.XY,
                )

            for g in range(NG):
                psums = [
                    [
                        psum_pool.tile([P, NFREE], f32, tag="ps", name=f"ps_{g}_{bb}_{o}")
                        for o in range(OT)
                    ]
                    for bb in range(GRP)
                ]
                if g < NG - 1:
                    # k-major over the group's 4 batch tiles: each wt k-slice
                    # is reused by 8 back-to-back matmuls the moment it
                    # lands, so group 0 streams from HBM without stalling.
                    # Pooling lands at the group tail and overlaps the next
                    # group's matmuls.
                    for k in range(KT):
                        last = k == KT - 1
                        for bb in range(GRP):
                            lhsT = xt_sb[(k, g)][:, bb * P : (bb + 1) * P]
                            for o in range(OT):
                                nc.tensor.matmul(
                                    psums[bb][o][:],
                                    lhsT,
                                    wt_sb[k][:, o * NFREE : (o + 1) * NFREE],
                                    start=(k == 0),
                                    stop=last,
                                )
                            if last:
                                emit_pooling(g, bb, psums)
                else:
                    # Last group: batch-tile-major so each tile's pooling
                    # overlaps the next tile's matmuls instead of piling up
                    # after the final matmul (everything is SBUF-resident by
                    # now, so wt reuse order no longer matters).
                    for bb in range(GRP):
                        for k in range(KT):
                            lhsT = xt_sb[(k, g)][:, bb * P : (bb + 1) * P]
                            for o in range(OT):
                                nc.tensor.matmul(
                                    psums[bb][o][:],
                                    lhsT,
                                    wt_sb[k][:, o * NFREE : (o + 1) * NFREE],
                                    start=(k == 0),
                                    stop=(k == KT - 1),
                                )
                        emit_pooling(g, bb, psums)

            nc.sync.dma_start(out_d[:, :], outsb[:])

    nc.compile()
    _dedup_ldweights(nc)
    return nc


def make_in_maps(x, W, b):
    """Host-side shard + preprocess: transpose, fold 0.5, cast bf16."""
    x = np.asarray(x, dtype=np.float32)
    W = np.asarray(W, dtype=np.float32)
    b = np.asarray(b, dtype=np.float32)

    xt = np.ascontiguousarray(x.T).astype(ml_dtypes.bfloat16)  # [I, B]
    wt = np.ascontiguousarray(W.T * np.float32(SCALE)).astype(
        ml_dtypes.bfloat16
    )  # [I, O]
    bias = (b * np.float32(SCALE)).reshape(1, O)

    # Per-batch-half x slabs: [KT, NG, P, GRP*P]
    x_slabs = []
    for g in range(NB_G):
        xg = xt[:, g * BC : (g + 1) * BC]  # [I, BC]
        xr = np.ascontiguousarray(
            xg.reshape(KT, P, NG, GRP * P).transpose(0, 2, 1, 3)
        )
        x_slabs.append(xr)
    # Per-out-feature-quarter W slabs [KT, P, OC] and replicated bias rows.
    w_slabs = []
    b_slabs = []
    for j in range(NO_G):
        w_slabs.append(
            np.ascontiguousarray(wt[:, j * OC : (j + 1) * OC]).reshape(KT, P, OC)
        )
        b_slabs.append(
            np.ascontiguousarray(
                np.broadcast_to(bias[:, j * OC : (j + 1) * OC], (P, OC))
            )
        )

    in_maps = []
    for c in range(NB_G * NO_G):
        g, j = divmod(c, NO_G)
        in_maps.append({"xt": x_slabs[g], "wt": w_slabs[j], "biasrep": b_slabs[j]})
    return in_maps


def combine_outputs(results):
    """Sum the 4 out-feature partials per batch half -> full [B] output."""
    out = np.zeros(B, dtype=np.float32)
    for c, r in enumerate(results):
        g = c // NO_G
        part = np.asarray(r["out"], dtype=np.float32)  # [P, BT]
        out[g * BC : (g + 1) * BC] += part.T.reshape(BC)
    return out


def kernel(x, W, b):
    from concourse.bass_utils import run_bass_kernel_spmd

    if "nc" not in _NC_CACHE:
        _NC_CACHE["nc"] = build_bass()
    nc = _NC_CACHE["nc"]
    in_maps = make_in_maps(x, W, b)
    res = run_bass_kernel_spmd(nc, in_maps, core_ids=list(range(NB_G * NO_G)))
    return combine_outputs(res.results)


# revision 21
# speedup vs baseline: 1.0021x; 1.0021x over previous
"""Trainium2 Bass kernel for: out = 0.5 * sum_g maxpool4(x @ W.T + b).

Shapes: x [4096, 2048] f32, W [4096, 2048] f32, b [4096] f32 -> out [4096] f32.

Sharding over 8 NeuronCores: 2 batch-groups x 4 out-feature-groups.
Core c = (g, j): batch rows g*2048:(g+1)*2048, out features j*1024:(j+1)*1024.
Each core computes partial row-sums of its pooled quarter; host adds the 4
out-feature partials per batch half (pooling groups of 4 are never split
across cores since 1024 % 4 == 0).

Per-core kernel: y tile layout [batch=128 partitions, out_f=512 free].
  lhsT (stationary) = x^T k-slice [128 i, 128 b], rhs (moving) = W^T k-slice
  [128 i, 512 o], accumulating over 16 k-slices into PSUM fp32. The bias add
  rides the VectorE pooling stage (bias+maxpool4 via strided adds + max
  tree), keeping the PE stream pure matmul. The 0.5 output scale is folded
  into W and b on the host (max is monotone under positive scaling). Inputs
  are cast to bf16 on host (PE runs bf16 at 1 cycle/row vs 4 for fp32); PSUM
  accumulation stays fp32 and the bias is added in fp32.

Loop order: k-major over groups of 4 batch-tiles (8 PSUM banks = 4 b x 2 o)
so each W^T k-slice DMA is consumed by 8 back-to-back matmuls and the PE
never waits on HBM after the first slice. Everything (~12.5 MiB/core) stays
resident in SBUF. The first (xt, wt) slices are DMA'd from GpSimd, which
exits the NEFF preamble ~1.3 us before Sync, shaving the first-matmul
latency; Sync streams the remaining W slices in parallel.
"""

import sys

if "/opt/trn_rl_repo" not in sys.path:
    sys.path.insert(0, "/opt/trn_rl_repo")

import numpy as np
import ml_dtypes

# Problem constants (hardcoded per harness contract).
B, I, O = 4096, 2048, 4096
KS = 4  # maxpool kernel size
SCALE = 0.5
NB_G, NO_G = 2, 4  # batch groups x out-feature groups = 8 cores
BC = B // NB_G  # 2048 batch rows per core
OC = O // NO_G  # 1024 out features per core
P = 128
KT = I // P  # 16 contraction slices
BT = BC // P  # 16 batch tiles per core
GRP = 4  # batch tiles per k-major group
NG = BT // GRP  # 4 groups
NFREE = 512  # matmul moving free dim (one PSUM bank fp32)
OT = OC // NFREE  # 2 out-feature tiles of 512 per core

_NC_CACHE = {}


def _dedup_ldweights(nc):
    """Remove redundant standalone Ldweights from the compiled module.

    bacc splits every Matmult into Ldweights + Matmult(ldweights=False) with
    no dedup, costing ~46 ns/matmul of PE queue time. When consecutive PE
    Ldweights load the identical stationary AP, the array already holds the
    weights, so sync-free duplicates can be dropped. Only duplicates with no
    semaphore waits/updates are removed (a wait-carrying Ldweights guards a
    real dependency).
    """
    removed = 0
    for f in nc.m.functions:
        for blk in f.blocks:
            insts = list(blk.instructions)
            keep = []
            blk_removed = 0
            last_key = None
            for ins in insts:
                tname = type(ins).__name__
                if tname == "InstLdweights":
                    ap = ins.ins[0]
                    key = (
                        ap.memref,
                        ap.offset,
                        str(ap.ap),
                        str(ap.dtype),
                        str(ins.tile_position),
                        str(ins.tile_size),
                        str(ins.perf_mode),
                        str(ins.is_transpose),
                    )
                    if (
                        key == last_key
                        and not ins.has_wait()
                        and not ins.has_update()
                    ):
                        blk_removed += 1
                        continue
                    last_key = key
                keep.append(ins)
            if blk_removed:
                blk.instructions[:] = keep
                removed += blk_removed
    return removed


def build_bass():
    """Build the (SPMD, per-core) Bass program."""
    from concourse import bacc, tile
    import concourse.mybir as mybir

    f32 = mybir.dt.float32
    bf16 = mybir.dt.bfloat16

    # Bacc (not plain Bass): its compile() runs the TRN2 legalization passes
    # (move_matmul_waits_to_ldweights, generate_event_semaphores) without
    # which walrus rejects matmuls carrying >1 semaphore wait.
    nc = bacc.Bacc(
        "TRN2", target_bir_lowering=False, debug=False, num_devices=NB_G * NO_G
    )
    xt_d = nc.dram_tensor("xt", [KT, NG, P, GRP * P], bf16, kind="ExternalInput")
    wt_d = nc.dram_tensor("wt", [KT, P, OC], bf16, kind="ExternalInput")
    biasrep_d = nc.dram_tensor("biasrep", [P, OC], f32, kind="ExternalInput")
    out_d = nc.dram_tensor("out", [P, BT], f32, kind="ExternalOutput")

    with tile.TileContext(nc) as tc:
        with (
            tc.tile_pool(name="wt", bufs=KT) as wt_pool,
            tc.tile_pool(name="xt", bufs=KT * NG) as xt_pool,
            tc.tile_pool(name="misc", bufs=1) as misc_pool,
            tc.tile_pool(name="pool4", bufs=8) as pool4_pool,
            tc.tile_pool(name="pooled", bufs=4) as pooled_pool,
            tc.tile_pool(name="psum", bufs=8, space="PSUM") as psum_pool,
        ):
            # wt streams on Sync (wt0 first — first matmul gates on it), xt
            # on GpSimd, so the two engines' descriptor prep and transfer
            # queues run in parallel.
            # PE warmup: the HAM clock gate keeps the PE at 1.2 GHz until it
            # has been busy ~3.4 us. The PE would otherwise idle 6.5-11 us
            # into the kernel waiting for the first DMA, then pay the cold
            # penalty on real matmuls. Run K=1 garbage matmuls (scratch SBUF,
            # overwritten PSUM) during the DMA wait so the real stream starts
            # warm.
            warm = misc_pool.tile([1, NFREE + P], bf16)
            nc.vector.memset(warm[:], 0.0)

            wt_sb = []
            xt_sb = {}
            # k=0 arrives in fine-grained pieces so the first matmul (which
            # gates the whole PE stream behind the ~7 us NEFF preamble) can
            # start after ~160 KB instead of ~384 KB: first the bb=0 x
            # columns and the o=0 W half, then the rest. The two critical
            # pieces ride the earliest-ready engine queues.
            w = wt_pool.tile([P, OC], bf16, tag="wt", name="w_0")
            xx = xt_pool.tile([P, GRP * P], bf16, tag="xt", name="xx_0_0")
            nc.sync.dma_start(xx[:, 0:P], xt_d[0, 0, :, 0:P])
            nc.sync.dma_start(w[:, 0:NFREE], wt_d[0, :, 0:NFREE])
            nc.gpsimd.dma_start(xx[:, P:], xt_d[0, 0, :, P:])
            nc.sync.dma_start(w[:, NFREE:], wt_d[0, :, NFREE:])
            wt_sb.append(w)
            xt_sb[(0, 0)] = xx

            biasrep = misc_pool.tile([P, OC], f32)
            nc.scalar.dma_start(biasrep[:], biasrep_d[:, :])
            outsb = misc_pool.tile([P, BT], f32)

            for k in range(1, KT):
                w = wt_pool.tile([P, OC], bf16, tag="wt", name=f"w_{k}")
                nc.sync.dma_start(w[:], wt_d[k, :, :])
                wt_sb.append(w)
                xx = xt_pool.tile([P, GRP * P], bf16, tag="xt", name=f"xx_{k}_0")
                nc.gpsimd.dma_start(xx[:], xt_d[k, 0, :, :])
                xt_sb[(k, 0)] = xx
            for g in range(1, NG):
                for k in range(KT):
                    xx = xt_pool.tile([P, GRP * P], bf16, tag="xt", name=f"xx_{k}_{g}")
                    nc.gpsimd.dma_start(xx[:], xt_d[k, g, :, :])
                    xt_sb[(k, g)] = xx

            def emit_pooling(g, bb, psums):
                """bias add + maxpool4 + row-sum for one finished batch tile."""
                pooled = pooled_pool.tile(
                    [P, OT, P], f32, tag="pooled", name=f"pooled_{g}_{bb}"
                )
                for o in range(OT):
                    # y+bias in one full-width add (frees the PSUM bank
                    # early), then maxpool4 via 3D-AP reduce.
                    tsum = pool4_pool.tile(
                        [P, NFREE], f32, tag="tsum", name=f"tsum_{g}_{bb}_{o}"
                    )
                    nc.vector.tensor_add(
                        tsum[:],
                        psums[bb][o][:],
                        biasrep[:, o * NFREE : (o + 1) * NFREE],
                    )
                    nc.vector.reduce_max(
                        pooled[:, o, :],
                        tsum[:].rearrange("p (q f) -> p q f", f=KS),
                        axis=mybir.AxisListType.X,
                    )
                col = g * GRP + bb
                nc.vector.reduce_sum(
                    outsb[:, col : col + 1],
                    pooled[:, :, :],
                    axis=mybir.AxisListType.XY,
                )

            warm_psum = None
            for g in range(NG):
                psums = [
                    [
                        psum_pool.tile([P, NFREE], f32, tag="ps", name=f"ps_{g}_{bb}_{o}")
                        for o in range(OT)
                    ]
                    for bb in range(GRP)
                ]
                if g == 0:
                    # Warmup matmuls into bank (0,0) — garbage results, then
                    # the real k=0 start=True matmul overwrites the bank.
                    warm_psum = psums[0][0]
                    for _ in range(8):
                        nc.tensor.matmul(
                            warm_psum[:],
                            warm[:, 0:P],
                            warm[:, P : P + NFREE],
                            start=True,
                            stop=True,
                            skip_group_check=True,
                        )
                if g < NG - 1:
                    # k-major over the group's 4 batch tiles: each wt k-slice
                    # is reused by 8 back-to-back matmuls the moment it
                    # lands, so group 0 streams from HBM without stalling.
                    # Pooling lands at the group tail and overlaps the next
                    # group's matmuls.
                    for k in range(KT):
                        last = k == KT - 1
                        for bb in range(GRP):
                            lhsT = xt_sb[(k, g)][:, bb * P : (bb + 1) * P]
                            for o in range(OT):
                                nc.tensor.matmul(
                                    psums[bb][o][:],
                                    lhsT,
                                    wt_sb[k][:, o * NFREE : (o + 1) * NFREE],
                                    start=(k == 0),
                                    stop=last,
                                )
                            if last:
                                emit_pooling(g, bb, psums)
                else:
                    # Last group: batch-tile-major so each tile's pooling
                    # overlaps the next tile's matmuls instead of piling up
                    # after the final matmul (everything is SBUF-resident by
                    # now, so wt reuse order no longer matters).
                    for bb in range(GRP):
                        for k in range(KT):
                            lhsT = xt_sb[(k, g)][:, bb * P : (bb + 1) * P]
                            for o in range(OT):
                                nc.tensor.matmul(
                                    psums[bb][o][:],
                                    lhsT,
                                    wt_sb[k][:, o * NFREE : (o + 1) * NFREE],
                                    start=(k == 0),
                                    stop=(k == KT - 1),
                                )
                        emit_pooling(g, bb, psums)

            nc.sync.dma_start(out_d[:, :], outsb[:])

    nc.compile()
    _dedup_ldweights(nc)
    return nc


def make_in_maps(x, W, b):
    """Host-side shard + preprocess: transpose, fold 0.5, cast bf16."""
    x = np.asarray(x, dtype=np.float32)
    W = np.asarray(W, dtype=np.float32)
    b = np.asarray(b, dtype=np.float32)

    xt = np.ascontiguousarray(x.T).astype(ml_dtypes.bfloat16)  # [I, B]
    wt = np.ascontiguousarray(W.T * np.float32(SCALE)).astype(
        ml_dtypes.bfloat16
    )  # [I, O]
    bias = (b * np.float32(SCALE)).reshape(1, O)

    # Per-batch-half x slabs: [KT, NG, P, GRP*P]
    x_slabs = []
    for g in range(NB_G):
        xg = xt[:, g * BC : (g + 1) * BC]  # [I, BC]
        xr = np.ascontiguousarray(
            xg.reshape(KT, P, NG, GRP * P).transpose(0, 2, 1, 3)
        )
        x_slabs.append(xr)
    # Per-out-feature-quarter W slabs [KT, P, OC] and replicated bias rows.
    w_slabs = []
    b_slabs = []
    for j in range(NO_G):
        w_slabs.append(
            np.ascontiguousarray(wt[:, j * OC : (j + 1) * OC]).reshape(KT, P, OC)
        )
        b_slabs.append(
            np.ascontiguousarray(
                np.broadcast_to(bias[:, j * OC : (j + 1) * OC], (P, OC))
            )
        )

    in_maps = []
    for c in range(NB_G * NO_G):
        g, j = divmod(c, NO_G)
        in_maps.append({"xt": x_slabs[g], "wt": w_slabs[j], "biasrep": b_slabs[j]})
    return in_maps


def combine_outputs(results):
    """Sum the 4 out-feature partials per batch half -> full [B] output."""
    out = np.zeros(B, dtype=np.float32)
    for c, r in enumerate(results):
        g = c // NO_G
        part = np.asarray(r["out"], dtype=np.float32)  # [P, BT]
        out[g * BC : (g + 1) * BC] += part.T.reshape(BC)
    return out


def kernel(x, W, b):
    from concourse.bass_utils import run_bass_kernel_spmd

    if "nc" not in _NC_CACHE:
        _NC_CACHE["nc"] = build_bass()
    nc = _NC_CACHE["nc"]
    in_maps = make_in_maps(x, W, b)
    res = run_bass_kernel_spmd(nc, in_maps, core_ids=list(range(NB_G * NO_G)))
    return combine_outputs(res.results)


# revision 24
# speedup vs baseline: 1.0175x; 1.0154x over previous
"""Trainium2 Bass kernel for: out = 0.5 * sum_g maxpool4(x @ W.T + b).

Shapes: x [4096, 2048] f32, W [4096, 2048] f32, b [4096] f32 -> out [4096] f32.

Sharding over 8 NeuronCores: 2 batch-groups x 4 out-feature-groups.
Core c = (g, j): batch rows g*2048:(g+1)*2048, out features j*1024:(j+1)*1024.
Each core computes partial row-sums of its pooled quarter; host adds the 4
out-feature partials per batch half (pooling groups of 4 are never split
across cores since 1024 % 4 == 0).

Per-core kernel: y tile layout [batch=128 partitions, out_f=512 free].
  lhsT (stationary) = x^T k-slice [128 i, 128 b], rhs (moving) = W^T k-slice
  [128 i, 512 o], accumulating over 16 k-slices into PSUM fp32. The bias add
  rides the VectorE pooling stage (bias+maxpool4 via strided adds + max
  tree), keeping the PE stream pure matmul. The 0.5 output scale is folded
  into W and b on the host (max is monotone under positive scaling). Inputs
  are cast to bf16 on host (PE runs bf16 at 1 cycle/row vs 4 for fp32); PSUM
  accumulation stays fp32 and the bias is added in fp32.

Loop order: k-major over groups of 4 batch-tiles (8 PSUM banks = 4 b x 2 o)
so each W^T k-slice DMA is consumed by 8 back-to-back matmuls and the PE
never waits on HBM after the first slice. Everything (~12.5 MiB/core) stays
resident in SBUF. The first (xt, wt) slices are DMA'd from GpSimd, which
exits the NEFF preamble ~1.3 us before Sync, shaving the first-matmul
latency; Sync streams the remaining W slices in parallel.
"""

import sys

if "/opt/trn_rl_repo" not in sys.path:
    sys.path.insert(0, "/opt/trn_rl_repo")

import numpy as np
import ml_dtypes

# Problem constants (hardcoded per harness contract).
B, I, O = 4096, 2048, 4096
KS = 4  # maxpool kernel size
SCALE = 0.5
NB_G, NO_G = 2, 4  # batch groups x out-feature groups = 8 cores
BC = B // NB_G  # 2048 batch rows per core
OC = O // NO_G  # 1024 out features per core
P = 128
KT = I // P  # 16 contraction slices
BT = BC // P  # 16 batch tiles per core
GRP = 4  # batch tiles per k-major group
NG = BT // GRP  # 4 groups
NFREE = 512  # matmul moving free dim (one PSUM bank fp32)
OT = OC // NFREE  # 2 out-feature tiles of 512 per core

_NC_CACHE = {}


def _dedup_ldweights(nc):
    """Remove redundant standalone Ldweights from the compiled module.

    bacc splits every Matmult into Ldweights + Matmult(ldweights=False) with
    no dedup, costing ~46 ns/matmul of PE queue time. When consecutive PE
    Ldweights load the identical stationary AP, the array already holds the
    weights, so sync-free duplicates can be dropped. Only duplicates with no
    semaphore waits/updates are removed (a wait-carrying Ldweights guards a
    real dependency).
    """
    removed = 0
    for f in nc.m.functions:
        for blk in f.blocks:
            insts = list(blk.instructions)
            keep = []
            blk_removed = 0
            last_key = None
            for ins in insts:
                tname = type(ins).__name__
                if tname == "InstLdweights":
                    ap = ins.ins[0]
                    key = (
                        ap.memref,
                        ap.offset,
                        str(ap.ap),
                        str(ap.dtype),
                        str(ins.tile_position),
                        str(ins.tile_size),
                        str(ins.perf_mode),
                        str(ins.is_transpose),
                    )
                    if (
                        key == last_key
                        and not ins.has_wait()
                        and not ins.has_update()
                    ):
                        blk_removed += 1
                        continue
                    last_key = key
                keep.append(ins)
            if blk_removed:
                blk.instructions[:] = keep
                removed += blk_removed
    return removed


def build_bass():
    """Build the (SPMD, per-core) Bass program."""
    from concourse import bacc, tile
    import concourse.mybir as mybir

    f32 = mybir.dt.float32
    bf16 = mybir.dt.bfloat16

    # Bacc (not plain Bass): its compile() runs the TRN2 legalization passes
    # (move_matmul_waits_to_ldweights, generate_event_semaphores) without
    # which walrus rejects matmuls carrying >1 semaphore wait.
    nc = bacc.Bacc(
        "TRN2",
        target_bir_lowering=False,
        debug=False,
        num_devices=NB_G * NO_G,
        enable_asserts=False,
        num_swdge_queues=2,
    )
    xt_d = nc.dram_tensor("xt", [KT, NG, P, GRP * P], bf16, kind="ExternalInput")
    wt_d = nc.dram_tensor("wt", [KT, P, OC], bf16, kind="ExternalInput")
    biasrep_d = nc.dram_tensor("biasrep", [P, OC], f32, kind="ExternalInput")
    out_d = nc.dram_tensor("out", [P, BT], f32, kind="ExternalOutput")

    with tile.TileContext(nc) as tc:
        with (
            tc.tile_pool(name="wt", bufs=KT) as wt_pool,
            tc.tile_pool(name="xt", bufs=KT * NG) as xt_pool,
            tc.tile_pool(name="misc", bufs=1) as misc_pool,
            tc.tile_pool(name="pool4", bufs=8) as pool4_pool,
            tc.tile_pool(name="pooled", bufs=4) as pooled_pool,
            tc.tile_pool(name="psum", bufs=8, space="PSUM") as psum_pool,
        ):
            # wt streams on Sync (wt0 first — first matmul gates on it), xt
            # on GpSimd, so the two engines' descriptor prep and transfer
            # queues run in parallel.
            wt_sb = []
            xt_sb = {}
            # k=0 arrives in fine-grained pieces so the first matmul (which
            # gates the whole PE stream behind the ~7 us NEFF preamble) can
            # start after ~160 KB instead of ~384 KB: first the bb=0 x
            # columns and the o=0 W half, then the rest.
            w = wt_pool.tile([P, OC], bf16, tag="wt", name="w_0")
            xx = xt_pool.tile([P, GRP * P], bf16, tag="xt", name="xx_0_0")
            nc.gpsimd.dma_start(xx[:, 0:P], xt_d[0, 0, :, 0:P])
            nc.sync.dma_start(w[:, 0:NFREE], wt_d[0, :, 0:NFREE])
            nc.gpsimd.dma_start(xx[:, P:], xt_d[0, 0, :, P:])
            nc.sync.dma_start(w[:, NFREE:], wt_d[0, :, NFREE:])
            wt_sb.append(w)
            xt_sb[(0, 0)] = xx

            biasrep = misc_pool.tile([P, OC], f32)
            nc.scalar.dma_start(biasrep[:], biasrep_d[:, :])
            outsb = misc_pool.tile([P, BT], f32)

            for k in range(1, KT):
                w = wt_pool.tile([P, OC], bf16, tag="wt", name=f"w_{k}")
                nc.sync.dma_start(w[:], wt_d[k, :, :])
                wt_sb.append(w)
                xx = xt_pool.tile([P, GRP * P], bf16, tag="xt", name=f"xx_{k}_0")
                nc.gpsimd.dma_start(xx[:], xt_d[k, 0, :, :])
                xt_sb[(k, 0)] = xx
            for g in range(1, NG):
                for k in range(KT):
                    xx = xt_pool.tile([P, GRP * P], bf16, tag="xt", name=f"xx_{k}_{g}")
                    nc.gpsimd.dma_start(xx[:], xt_d[k, g, :, :])
                    xt_sb[(k, g)] = xx

            def emit_pooling(g, bb, psums):
                """bias add + maxpool4 + row-sum for one finished batch tile."""
                pooled = pooled_pool.tile(
                    [P, OT, P], f32, tag="pooled", name=f"pooled_{g}_{bb}"
                )
                for o in range(OT):
                    # y+bias in one full-width add (frees the PSUM bank
                    # early), then maxpool4 via 3D-AP reduce.
                    tsum = pool4_pool.tile(
                        [P, NFREE], f32, tag="tsum", name=f"tsum_{g}_{bb}_{o}"
                    )
                    nc.vector.tensor_add(
                        tsum[:],
                        psums[bb][o][:],
                        biasrep[:, o * NFREE : (o + 1) * NFREE],
                    )
                    nc.vector.reduce_max(
                        pooled[:, o, :],
                        tsum[:].rearrange("p (q f) -> p q f", f=KS),
                        axis=mybir.AxisListType.X,
                    )
                col = g * GRP + bb
                nc.vector.reduce_sum(
                    outsb[:, col : col + 1],
                    pooled[:, :, :],
                    axis=mybir.AxisListType.XY,
                )

            for g in range(NG):
                psums = [
                    [
                        psum_pool.tile([P, NFREE], f32, tag="ps", name=f"ps_{g}_{bb}_{o}")
                        for o in range(OT)
                    ]
                    for bb in range(GRP)
                ]
                if g < NG - 1:
                    # k-major over the group's 4 batch tiles: each wt k-slice
                    # is reused by 8 back-to-back matmuls the moment it
                    # lands, so group 0 streams from HBM without stalling.
                    # Pooling lands at the group tail and overlaps the next
                    # group's matmuls.
                    for k in range(KT):
                        last = k == KT - 1
                        for bb in range(GRP):
                            lhsT = xt_sb[(k, g)][:, bb * P : (bb + 1) * P]
                            for o in range(OT):
                                nc.tensor.matmul(
                                    psums[bb][o][:],
                                    lhsT,
                                    wt_sb[k][:, o * NFREE : (o + 1) * NFREE],
                                    start=(k == 0),
                                    stop=last,
                                )
                            if last:
                                emit_pooling(g, bb, psums)
                else:
                    # Last group: batch-tile-major so each tile's pooling
                    # overlaps the next tile's matmuls instead of piling up
                    # after the final matmul (everything is SBUF-resident by
                    # now, so wt reuse order no longer matters).
                    for bb in range(GRP):
                        for k in range(KT):
                            lhsT = xt_sb[(k, g)][:, bb * P : (bb + 1) * P]
                            for o in range(OT):
                                nc.tensor.matmul(
                                    psums[bb][o][:],
                                    lhsT,
                                    wt_sb[k][:, o * NFREE : (o + 1) * NFREE],
                                    start=(k == 0),
                                    stop=(k == KT - 1),
                                )
                        emit_pooling(g, bb, psums)

            nc.sync.dma_start(out_d[:, :], outsb[:])

    nc.compile()
    _dedup_ldweights(nc)
    return nc


def make_in_maps(x, W, b):
    """Host-side shard + preprocess: transpose, fold 0.5, cast bf16."""
    x = np.asarray(x, dtype=np.float32)
    W = np.asarray(W, dtype=np.float32)
    b = np.asarray(b, dtype=np.float32)

    xt = np.ascontiguousarray(x.T).astype(ml_dtypes.bfloat16)  # [I, B]
    wt = np.ascontiguousarray(W.T * np.float32(SCALE)).astype(
        ml_dtypes.bfloat16
    )  # [I, O]
    bias = (b * np.float32(SCALE)).reshape(1, O)

    # Per-batch-half x slabs: [KT, NG, P, GRP*P]
    x_slabs = []
    for g in range(NB_G):
        xg = xt[:, g * BC : (g + 1) * BC]  # [I, BC]
        xr = np.ascontiguousarray(
            xg.reshape(KT, P, NG, GRP * P).transpose(0, 2, 1, 3)
        )
        x_slabs.append(xr)
    # Per-out-feature-quarter W slabs [KT, P, OC] and replicated bias rows.
    w_slabs = []
    b_slabs = []
    for j in range(NO_G):
        w_slabs.append(
            np.ascontiguousarray(wt[:, j * OC : (j + 1) * OC]).reshape(KT, P, OC)
        )
        b_slabs.append(
            np.ascontiguousarray(
                np.broadcast_to(bias[:, j * OC : (j + 1) * OC], (P, OC))
            )
        )

    in_maps = []
    for c in range(NB_G * NO_G):
        g, j = divmod(c, NO_G)
        in_maps.append({"xt": x_slabs[g], "wt": w_slabs[j], "biasrep": b_slabs[j]})
    return in_maps


def combine_outputs(results):
    """Sum the 4 out-feature partials per batch half -> full [B] output."""
    out = np.zeros(B, dtype=np.float32)
    for c, r in enumerate(results):
        g = c // NO_G
        part = np.asarray(r["out"], dtype=np.float32)  # [P, BT]
        out[g * BC : (g + 1) * BC] += part.T.reshape(BC)
    return out


def kernel(x, W, b):
    from concourse.bass_utils import run_bass_kernel_spmd

    if "nc" not in _NC_CACHE:
        _NC_CACHE["nc"] = build_bass()
    nc = _NC_CACHE["nc"]
    in_maps = make_in_maps(x, W, b)
    res = run_bass_kernel_spmd(nc, in_maps, core_ids=list(range(NB_G * NO_G)))
    return combine_outputs(res.results)


# revision 26
# speedup vs baseline: 1.0183x; 1.0008x over previous
"""Trainium2 Bass kernel for: out = 0.5 * sum_g maxpool4(x @ W.T + b).

Shapes: x [4096, 2048] f32, W [4096, 2048] f32, b [4096] f32 -> out [4096] f32.

Sharding over 8 NeuronCores: 2 batch-groups x 4 out-feature-groups.
Core c = (g, j): batch rows g*2048:(g+1)*2048, out features j*1024:(j+1)*1024.
Each core computes partial row-sums of its pooled quarter; host adds the 4
out-feature partials per batch half (pooling groups of 4 are never split
across cores since 1024 % 4 == 0).

Per-core kernel: y tile layout [batch=128 partitions, out_f=512 free].
  lhsT (stationary) = x^T k-slice [128 i, 128 b], rhs (moving) = W^T k-slice
  [128 i, 512 o], accumulating over 16 k-slices into PSUM fp32. The bias add
  rides the VectorE pooling stage (bias+maxpool4 via strided adds + max
  tree), keeping the PE stream pure matmul. The 0.5 output scale is folded
  into W and b on the host (max is monotone under positive scaling). Inputs
  are cast to bf16 on host (PE runs bf16 at 1 cycle/row vs 4 for fp32); PSUM
  accumulation stays fp32 and the bias is added in fp32.

Loop order: k-major over groups of 4 batch-tiles (8 PSUM banks = 4 b x 2 o)
so each W^T k-slice DMA is consumed by 8 back-to-back matmuls and the PE
never waits on HBM after the first slice. Everything (~12.5 MiB/core) stays
resident in SBUF. The first (xt, wt) slices are DMA'd from GpSimd, which
exits the NEFF preamble ~1.3 us before Sync, shaving the first-matmul
latency; Sync streams the remaining W slices in parallel.
"""

import sys

if "/opt/trn_rl_repo" not in sys.path:
    sys.path.insert(0, "/opt/trn_rl_repo")

import numpy as np
import ml_dtypes

# Problem constants (hardcoded per harness contract).
B, I, O = 4096, 2048, 4096
KS = 4  # maxpool kernel size
SCALE = 0.5
NB_G, NO_G = 2, 4  # batch groups x out-feature groups = 8 cores
BC = B // NB_G  # 2048 batch rows per core
OC = O // NO_G  # 1024 out features per core
P = 128
KT = I // P  # 16 contraction slices
BT = BC // P  # 16 batch tiles per core
GRP = 4  # batch tiles per k-major group
NG = BT // GRP  # 4 groups
NFREE = 512  # matmul moving free dim (one PSUM bank fp32)
OT = OC // NFREE  # 2 out-feature tiles of 512 per core

_NC_CACHE = {}


def _dedup_ldweights(nc):
    """Remove redundant standalone Ldweights from the compiled module.

    bacc splits every Matmult into Ldweights + Matmult(ldweights=False) with
    no dedup, costing ~46 ns/matmul of PE queue time. When consecutive PE
    Ldweights load the identical stationary AP, the array already holds the
    weights, so sync-free duplicates can be dropped. Only duplicates with no
    semaphore waits/updates are removed (a wait-carrying Ldweights guards a
    real dependency).
    """
    removed = 0
    for f in nc.m.functions:
        for blk in f.blocks:
            insts = list(blk.instructions)
            keep = []
            blk_removed = 0
            last_key = None
            for ins in insts:
                tname = type(ins).__name__
                if tname == "InstLdweights":
                    ap = ins.ins[0]
                    key = (
                        ap.memref,
                        ap.offset,
                        str(ap.ap),
                        str(ap.dtype),
                        str(ins.tile_position),
                        str(ins.tile_size),
                        str(ins.perf_mode),
                        str(ins.is_transpose),
                    )
                    if (
                        key == last_key
                        and not ins.has_wait()
                        and not ins.has_update()
                    ):
                        blk_removed += 1
                        continue
                    last_key = key
                keep.append(ins)
            if blk_removed:
                blk.instructions[:] = keep
                removed += blk_removed
    return removed


def build_bass():
    """Build the (SPMD, per-core) Bass program."""
    from concourse import bacc, tile
    import concourse.mybir as mybir

    f32 = mybir.dt.float32
    bf16 = mybir.dt.bfloat16

    # Bacc (not plain Bass): its compile() runs the TRN2 legalization passes
    # (move_matmul_waits_to_ldweights, generate_event_semaphores) without
    # which walrus rejects matmuls carrying >1 semaphore wait.
    nc = bacc.Bacc(
        "TRN2",
        target_bir_lowering=False,
        debug=False,
        num_devices=NB_G * NO_G,
        enable_asserts=False,
        num_swdge_queues=2,
    )
    xt_d = nc.dram_tensor("xt", [KT, NG, P, GRP * P], bf16, kind="ExternalInput")
    wt_d = nc.dram_tensor("wt", [KT, P, OC], bf16, kind="ExternalInput")
    biasrep_d = nc.dram_tensor("biasrep", [P, OC], f32, kind="ExternalInput")
    out_d = nc.dram_tensor("out", [P, BT], f32, kind="ExternalOutput")

    with tile.TileContext(nc) as tc:
        with (
            tc.tile_pool(name="wt", bufs=KT) as wt_pool,
            tc.tile_pool(name="xt", bufs=KT * NG) as xt_pool,
            tc.tile_pool(name="misc", bufs=1) as misc_pool,
            tc.tile_pool(name="pool4", bufs=8) as pool4_pool,
            tc.tile_pool(name="pooled", bufs=4) as pooled_pool,
            tc.tile_pool(name="psum", bufs=8, space="PSUM") as psum_pool,
        ):
            # wt streams on Sync (wt0 first — first matmul gates on it), xt
            # on GpSimd, so the two engines' descriptor prep and transfer
            # queues run in parallel.
            wt_sb = []
            xt_sb = {}
            # k=0 arrives in fine-grained pieces so the first matmul (which
            # gates the whole PE stream behind the ~7 us NEFF preamble) can
            # start after ~160 KB instead of ~384 KB: first the bb=0 x
            # columns and the o=0 W half, then the rest.
            w = wt_pool.tile([P, OC], bf16, tag="wt", name="w_0")
            xx = xt_pool.tile([P, GRP * P], bf16, tag="xt", name="xx_0_0")
            nc.gpsimd.dma_start(xx[:, 0:P], xt_d[0, 0, :, 0:P])
            nc.sync.dma_start(w[:, 0:NFREE], wt_d[0, :, 0:NFREE])
            nc.gpsimd.dma_start(xx[:, P:], xt_d[0, 0, :, P:])
            nc.sync.dma_start(w[:, NFREE:], wt_d[0, :, NFREE:])
            wt_sb.append(w)
            xt_sb[(0, 0)] = xx

            biasrep = misc_pool.tile([P, OC], f32)
            nc.scalar.dma_start(biasrep[:], biasrep_d[:, :])
            outsb = misc_pool.tile([P, BT], f32)

            for k in range(1, KT):
                w = wt_pool.tile([P, OC], bf16, tag="wt", name=f"w_{k}")
                nc.sync.dma_start(w[:], wt_d[k, :, :])
                wt_sb.append(w)
                xx = xt_pool.tile([P, GRP * P], bf16, tag="xt", name=f"xx_{k}_0")
                nc.gpsimd.dma_start(xx[:], xt_d[k, 0, :, :])
                xt_sb[(k, 0)] = xx
            for g in range(1, NG):
                for k in range(KT):
                    xx = xt_pool.tile([P, GRP * P], bf16, tag="xt", name=f"xx_{k}_{g}")
                    nc.gpsimd.dma_start(xx[:], xt_d[k, g, :, :])
                    xt_sb[(k, g)] = xx

            def emit_pooling(g, bb, psums):
                """bias add + maxpool4 + row-sum for one finished batch tile."""
                pooled = pooled_pool.tile(
                    [P, OT, P], f32, tag="pooled", name=f"pooled_{g}_{bb}"
                )
                for o in range(OT):
                    # y+bias in one full-width add (frees the PSUM bank
                    # early), then maxpool4 via 3D-AP reduce.
                    tsum = pool4_pool.tile(
                        [P, NFREE], f32, tag="tsum", name=f"tsum_{g}_{bb}_{o}"
                    )
                    nc.vector.tensor_add(
                        tsum[:],
                        psums[bb][o][:],
                        biasrep[:, o * NFREE : (o + 1) * NFREE],
                    )
                    nc.vector.reduce_max(
                        pooled[:, o, :],
                        tsum[:].rearrange("p (q f) -> p q f", f=KS),
                        axis=mybir.AxisListType.X,
                    )
                col = g * GRP + bb
                nc.vector.reduce_sum(
                    outsb[:, col : col + 1],
                    pooled[:, :, :],
                    axis=mybir.AxisListType.XY,
                )

            for g in range(NG):
                psums = [
                    [
                        psum_pool.tile([P, NFREE], f32, tag="ps", name=f"ps_{g}_{bb}_{o}")
                        for o in range(OT)
                    ]
                    for bb in range(GRP)
                ]
                if g < NG - 1:
                    # k-major over the group's 4 batch tiles: each wt k-slice
                    # is reused by 8 back-to-back matmuls the moment it
                    # lands, so group 0 streams from HBM without stalling.
                    # Pooling lands at the group tail and overlaps the next
                    # group's matmuls.
                    for k in range(KT):
                        last = k == KT - 1
                        for bb in range(GRP):
                            lhsT = xt_sb[(k, g)][:, bb * P : (bb + 1) * P]
                            for o in range(OT):
                                nc.tensor.matmul(
                                    psums[bb][o][:],
                                    lhsT,
                                    wt_sb[k][:, o * NFREE : (o + 1) * NFREE],
                                    start=(k == 0),
                                    stop=last,
                                )
                            if last:
                                emit_pooling(g, bb, psums)
                else:
                    # Last group: batch-tile-major so each tile's pooling
                    # overlaps the next tile's matmuls instead of piling up
                    # after the final matmul (everything is SBUF-resident by
                    # now, so wt reuse order no longer matters).
                    for bb in range(GRP):
                        for k in range(KT):
                            lhsT = xt_sb[(k, g)][:, bb * P : (bb + 1) * P]
                            for o in range(OT):
                                nc.tensor.matmul(
                                    psums[bb][o][:],
                                    lhsT,
                                    wt_sb[k][:, o * NFREE : (o + 1) * NFREE],
                                    start=(k == 0),
                                    stop=(k == KT - 1),
                                )
                        emit_pooling(g, bb, psums)

            # Output DMA from ScalarE — idle since its biasrep load, so the
            # descriptor prep isn't queued behind Sync/GpSimd streams.
            nc.scalar.dma_start(out_d[:, :], outsb[:])

    nc.compile()
    _dedup_ldweights(nc)
    return nc


def make_in_maps(x, W, b):
    """Host-side shard + preprocess: transpose, fold 0.5, cast bf16."""
    x = np.asarray(x, dtype=np.float32)
    W = np.asarray(W, dtype=np.float32)
    b = np.asarray(b, dtype=np.float32)

    xt = np.ascontiguousarray(x.T).astype(ml_dtypes.bfloat16)  # [I, B]
    wt = np.ascontiguousarray(W.T * np.float32(SCALE)).astype(
        ml_dtypes.bfloat16
    )  # [I, O]
    bias = (b * np.float32(SCALE)).reshape(1, O)

    # Per-batch-half x slabs: [KT, NG, P, GRP*P]
    x_slabs = []
    for g in range(NB_G):
        xg = xt[:, g * BC : (g + 1) * BC]  # [I, BC]
        xr = np.ascontiguousarray(
            xg.reshape(KT, P, NG, GRP * P).transpose(0, 2, 1, 3)
        )
        x_slabs.append(xr)
    # Per-out-feature-quarter W slabs [KT, P, OC] and replicated bias rows.
    w_slabs = []
    b_slabs = []
    for j in range(NO_G):
        w_slabs.append(
            np.ascontiguousarray(wt[:, j * OC : (j + 1) * OC]).reshape(KT, P, OC)
        )
        b_slabs.append(
            np.ascontiguousarray(
                np.broadcast_to(bias[:, j * OC : (j + 1) * OC], (P, OC))
            )
        )

    in_maps = []
    for c in range(NB_G * NO_G):
        g, j = divmod(c, NO_G)
        in_maps.append({"xt": x_slabs[g], "wt": w_slabs[j], "biasrep": b_slabs[j]})
    return in_maps


def combine_outputs(results):
    """Sum the 4 out-feature partials per batch half -> full [B] output."""
    out = np.zeros(B, dtype=np.float32)
    for c, r in enumerate(results):
        g = c // NO_G
        part = np.asarray(r["out"], dtype=np.float32)  # [P, BT]
        out[g * BC : (g + 1) * BC] += part.T.reshape(BC)
    return out


def kernel(x, W, b):
    from concourse.bass_utils import run_bass_kernel_spmd

    if "nc" not in _NC_CACHE:
        _NC_CACHE["nc"] = build_bass()
    nc = _NC_CACHE["nc"]
    in_maps = make_in_maps(x, W, b)
    res = run_bass_kernel_spmd(nc, in_maps, core_ids=list(range(NB_G * NO_G)))
    return combine_outputs(res.results)


# revision 30
# speedup vs baseline: 1.0228x; 1.0044x over previous
"""Trainium2 Bass kernel for: out = 0.5 * sum_g maxpool4(x @ W.T + b).

Shapes: x [4096, 2048] f32, W [4096, 2048] f32, b [4096] f32 -> out [4096] f32.

Sharding over 8 NeuronCores: 2 batch-groups x 4 out-feature-groups.
Core c = (g, j): batch rows g*2048:(g+1)*2048, out features j*1024:(j+1)*1024.
Each core computes partial row-sums of its pooled quarter; host adds the 4
out-feature partials per batch half (pooling groups of 4 are never split
across cores since 1024 % 4 == 0).

Per-core kernel: y tile layout [batch=128 partitions, out_f=512 free].
  lhsT (stationary) = x^T k-slice [128 i, 128 b], rhs (moving) = W^T k-slice
  [128 i, 512 o], accumulating over 16 k-slices into PSUM fp32. The bias add
  rides the VectorE pooling stage (bias+maxpool4 via strided adds + max
  tree), keeping the PE stream pure matmul. The 0.5 output scale is folded
  into W and b on the host (max is monotone under positive scaling). Inputs
  are cast to bf16 on host (PE runs bf16 at 1 cycle/row vs 4 for fp32); PSUM
  accumulation stays fp32 and the bias is added in fp32.

Loop order: k-major over groups of 4 batch-tiles (8 PSUM banks = 4 b x 2 o)
so each W^T k-slice DMA is consumed by 8 back-to-back matmuls and the PE
never waits on HBM after the first slice. Everything (~12.5 MiB/core) stays
resident in SBUF. The first (xt, wt) slices are DMA'd from GpSimd, which
exits the NEFF preamble ~1.3 us before Sync, shaving the first-matmul
latency; Sync streams the remaining W slices in parallel.
"""

import sys

if "/opt/trn_rl_repo" not in sys.path:
    sys.path.insert(0, "/opt/trn_rl_repo")

import numpy as np
import ml_dtypes

# Problem constants (hardcoded per harness contract).
B, I, O = 4096, 2048, 4096
KS = 4  # maxpool kernel size
SCALE = 0.5
NB_G, NO_G = 2, 4  # batch groups x out-feature groups = 8 cores
BC = B // NB_G  # 2048 batch rows per core
OC = O // NO_G  # 1024 out features per core
P = 128
KT = I // P  # 16 contraction slices
BT = BC // P  # 16 batch tiles per core
GRP = 4  # batch tiles per k-major group
NG = BT // GRP  # 4 groups
NFREE = 512  # matmul moving free dim (one PSUM bank fp32)
OT = OC // NFREE  # 2 out-feature tiles of 512 per core

_NC_CACHE = {}


def _dedup_ldweights(nc):
    """Remove redundant standalone Ldweights from the compiled module.

    bacc splits every Matmult into Ldweights + Matmult(ldweights=False) with
    no dedup, costing ~46 ns/matmul of PE queue time. When consecutive PE
    Ldweights load the identical stationary AP, the array already holds the
    weights, so sync-free duplicates can be dropped. Only duplicates with no
    semaphore waits/updates are removed (a wait-carrying Ldweights guards a
    real dependency).
    """
    removed = 0
    for f in nc.m.functions:
        for blk in f.blocks:
            insts = list(blk.instructions)
            keep = []
            blk_removed = 0
            last_key = None
            for ins in insts:
                tname = type(ins).__name__
                if tname == "InstLdweights":
                    ap = ins.ins[0]
                    key = (
                        ap.memref,
                        ap.offset,
                        str(ap.ap),
                        str(ap.dtype),
                        str(ins.tile_position),
                        str(ins.tile_size),
                        str(ins.perf_mode),
                        str(ins.is_transpose),
                    )
                    if (
                        key == last_key
                        and not ins.has_wait()
                        and not ins.has_update()
                    ):
                        blk_removed += 1
                        continue
                    last_key = key
                keep.append(ins)
            if blk_removed:
                blk.instructions[:] = keep
                removed += blk_removed
    return removed


def build_bass():
    """Build the (SPMD, per-core) Bass program."""
    from concourse import bacc, tile
    import concourse.mybir as mybir

    f32 = mybir.dt.float32
    bf16 = mybir.dt.bfloat16

    # Bacc (not plain Bass): its compile() runs the TRN2 legalization passes
    # (move_matmul_waits_to_ldweights, generate_event_semaphores) without
    # which walrus rejects matmuls carrying >1 semaphore wait.
    nc = bacc.Bacc(
        "TRN2",
        target_bir_lowering=False,
        debug=False,
        num_devices=NB_G * NO_G,
        enable_asserts=False,
        num_swdge_queues=2,
    )
    xt_d = nc.dram_tensor("xt", [KT, NG, P, GRP * P], bf16, kind="ExternalInput")
    wt_d = nc.dram_tensor("wt", [KT, P, OC], bf16, kind="ExternalInput")
    biasrep_d = nc.dram_tensor("biasrep", [P, OC], f32, kind="ExternalInput")
    out_d = nc.dram_tensor("out", [NG, P, GRP], f32, kind="ExternalOutput")

    with tile.TileContext(nc) as tc:
        with (
            tc.tile_pool(name="wt", bufs=KT) as wt_pool,
            tc.tile_pool(name="xt", bufs=KT * NG) as xt_pool,
            tc.tile_pool(name="misc", bufs=1) as misc_pool,
            tc.tile_pool(name="pool4", bufs=8) as pool4_pool,
            tc.tile_pool(name="pooled", bufs=4) as pooled_pool,
            tc.tile_pool(name="psum", bufs=8, space="PSUM") as psum_pool,
        ):
            # wt streams on Sync (wt0 first — first matmul gates on it), xt
            # on GpSimd, so the two engines' descriptor prep and transfer
            # queues run in parallel.
            wt_sb = []
            xt_sb = {}
            # k=0 arrives in fine-grained pieces so the first matmul (which
            # gates the whole PE stream behind the ~7 us NEFF preamble) can
            # start after ~160 KB instead of ~384 KB: first the bb=0 x
            # columns and the o=0 W half, then the rest.
            # The two pieces gating the first matmul ride two different
            # engine queues so their DMA first-byte latencies overlap.
            w = wt_pool.tile([P, OC], bf16, tag="wt", name="w_0")
            xx = xt_pool.tile([P, GRP * P], bf16, tag="xt", name="xx_0_0")
            nc.scalar.dma_start(xx[:, 0:P], xt_d[0, 0, :, 0:P])
            nc.sync.dma_start(w[:, 0:NFREE], wt_d[0, :, 0:NFREE])
            nc.gpsimd.dma_start(xx[:, P:], xt_d[0, 0, :, P:])
            nc.sync.dma_start(w[:, NFREE:], wt_d[0, :, NFREE:])
            wt_sb.append(w)
            xt_sb[(0, 0)] = xx

            biasrep = misc_pool.tile([P, OC], f32)
            nc.scalar.dma_start(biasrep[:], biasrep_d[:, :])
            outsb = misc_pool.tile([P, BT], f32)

            for k in range(1, KT):
                w = wt_pool.tile([P, OC], bf16, tag="wt", name=f"w_{k}")
                nc.sync.dma_start(w[:], wt_d[k, :, :])
                wt_sb.append(w)
                xx = xt_pool.tile([P, GRP * P], bf16, tag="xt", name=f"xx_{k}_0")
                nc.gpsimd.dma_start(xx[:], xt_d[k, 0, :, :])
                xt_sb[(k, 0)] = xx
            for g in range(1, NG):
                for k in range(KT):
                    xx = xt_pool.tile([P, GRP * P], bf16, tag="xt", name=f"xx_{k}_{g}")
                    nc.gpsimd.dma_start(xx[:], xt_d[k, g, :, :])
                    xt_sb[(k, g)] = xx

            def emit_pooling(g, bb, psums):
                """bias add + maxpool4 + row-sum for one finished batch tile."""
                pooled = pooled_pool.tile(
                    [P, OT, P], f32, tag="pooled", name=f"pooled_{g}_{bb}"
                )
                for o in range(OT):
                    # y+bias in one full-width add (frees the PSUM bank
                    # early), then maxpool4 via 3D-AP reduce.
                    tsum = pool4_pool.tile(
                        [P, NFREE], f32, tag="tsum", name=f"tsum_{g}_{bb}_{o}"
                    )
                    nc.vector.tensor_add(
                        tsum[:],
                        psums[bb][o][:],
                        biasrep[:, o * NFREE : (o + 1) * NFREE],
                    )
                    nc.vector.reduce_max(
                        pooled[:, o, :],
                        tsum[:].rearrange("p (q f) -> p q f", f=KS),
                        axis=mybir.AxisListType.X,
                    )
                col = g * GRP + bb
                nc.vector.reduce_sum(
                    outsb[:, col : col + 1],
                    pooled[:, :, :],
                    axis=mybir.AxisListType.XY,
                )

            for g in range(NG):
                psums = [
                    [
                        psum_pool.tile([P, NFREE], f32, tag="ps", name=f"ps_{g}_{bb}_{o}")
                        for o in range(OT)
                    ]
                    for bb in range(GRP)
                ]
                if g < NG - 1:
                    # k-major over the group's 4 batch tiles: each wt k-slice
                    # is reused by 8 back-to-back matmuls the moment it
                    # lands, so group 0 streams from HBM without stalling.
                    # Pooling lands at the group tail and overlaps the next
                    # group's matmuls.
                    for k in range(KT):
                        last = k == KT - 1
                        for bb in range(GRP):
                            lhsT = xt_sb[(k, g)][:, bb * P : (bb + 1) * P]
                            for o in range(OT):
                                nc.tensor.matmul(
                                    psums[bb][o][:],
                                    lhsT,
                                    wt_sb[k][:, o * NFREE : (o + 1) * NFREE],
                                    start=(k == 0),
                                    stop=last,
                                )
                            if last:
                                emit_pooling(g, bb, psums)
                else:
                    # Last group: batch-tile-major so each tile's pooling
                    # overlaps the next tile's matmuls instead of piling up
                    # after the final matmul (everything is SBUF-resident by
                    # now, so wt reuse order no longer matters).
                    for bb in range(GRP):
                        for k in range(KT):
                            lhsT = xt_sb[(k, g)][:, bb * P : (bb + 1) * P]
                            for o in range(OT):
                                nc.tensor.matmul(
                                    psums[bb][o][:],
                                    lhsT,
                                    wt_sb[k][:, o * NFREE : (o + 1) * NFREE],
                                    start=(k == 0),
                                    stop=(k == KT - 1),
                                )
                        emit_pooling(g, bb, psums)
                # Per-group output DMA (contiguous 2 KB in DRAM) so only the
                # last group's small piece sits after the final reduce; from
                # ScalarE, idle since its biasrep load.
                nc.scalar.dma_start(
                    out_d[g, :, :], outsb[:, g * GRP : (g + 1) * GRP]
                )

    nc.compile()
    _dedup_ldweights(nc)
    return nc


def make_in_maps(x, W, b):
    """Host-side shard + preprocess: transpose, fold 0.5, cast bf16."""
    x = np.asarray(x, dtype=np.float32)
    W = np.asarray(W, dtype=np.float32)
    b = np.asarray(b, dtype=np.float32)

    xt = np.ascontiguousarray(x.T).astype(ml_dtypes.bfloat16)  # [I, B]
    wt = np.ascontiguousarray(W.T * np.float32(SCALE)).astype(
        ml_dtypes.bfloat16
    )  # [I, O]
    bias = (b * np.float32(SCALE)).reshape(1, O)

    # Per-batch-half x slabs: [KT, NG, P, GRP*P]
    x_slabs = []
    for g in range(NB_G):
        xg = xt[:, g * BC : (g + 1) * BC]  # [I, BC]
        xr = np.ascontiguousarray(
            xg.reshape(KT, P, NG, GRP * P).transpose(0, 2, 1, 3)
        )
        x_slabs.append(xr)
    # Per-out-feature-quarter W slabs [KT, P, OC] and replicated bias rows.
    w_slabs = []
    b_slabs = []
    for j in range(NO_G):
        w_slabs.append(
            np.ascontiguousarray(wt[:, j * OC : (j + 1) * OC]).reshape(KT, P, OC)
        )
        b_slabs.append(
            np.ascontiguousarray(
                np.broadcast_to(bias[:, j * OC : (j + 1) * OC], (P, OC))
            )
        )

    in_maps = []
    for c in range(NB_G * NO_G):
        g, j = divmod(c, NO_G)
        in_maps.append({"xt": x_slabs[g], "wt": w_slabs[j], "biasrep": b_slabs[j]})
    return in_maps


def combine_outputs(results):
    """Sum the 4 out-feature partials per batch half -> full [B] output."""
    out = np.zeros(B, dtype=np.float32)
    for c, r in enumerate(results):
        g = c // NO_G
        part = np.asarray(r["out"], dtype=np.float32)  # [NG, P, GRP]
        # batch index within the core = (grp*GRP + bb)*P + p
        out[g * BC : (g + 1) * BC] += part.transpose(0, 2, 1).reshape(BC)
    return out


def kernel(x, W, b):
    from concourse.bass_utils import run_bass_kernel_spmd

    if "nc" not in _NC_CACHE:
        _NC_CACHE["nc"] = build_bass()
    nc = _NC_CACHE["nc"]
    in_maps = make_in_maps(x, W, b)
    res = run_bass_kernel_spmd(nc, in_maps, core_ids=list(range(NB_G * NO_G)))
    return combine_outputs(res.results)
